# revision 26
# baseline (speedup 1.0000x reference)
"""Trainium2 Bass kernel for CenterGeoAttention (N=65536, D=1024, H=16).

Row-shard N across 8 cores; fp8 DoubleRow matmuls; host-folded algebra.

Host precomputes (all cheap, O(N*D) or one D*D matmul):
  - LN stats r=1/sd, m per node; logits fold Wkp=(Wk@Qdiag)*gamma*256 (fp8),
    mncgb[h,n] = -256*cg_h*m_n, rb[h,n]=r_n (bf16, 16 rows).
  - h_aug[n] = r_n*[h | 32m | sd | 0pad] (fp8, 1040 cols) so ONE G matmul
    accumulates [G | 32*PRM | S] for the softmax partial sums.
  - a0c/g0c = W1b^T h_c + b1 etc; WA=Wo@W1b, WG=Wo@Wgb shorten the
    post-collective chain to Gn -> ocv -> a0/g0.
  - All streamed tensors pre-tiled so each chunk is ONE DMA with >=4KB
    per-partition lines (fp8 512B lines otherwise shatter into 45k
    descriptors).

Device, pass 1 (single sweep, 16 chunks): L = Wkp^T h (fp8 DR) + bias-proj
  L2; p = exp(r*(L-cg*m)/256 + L2 + cb); G += p^T h_aug (fp8 DR).
AllReduce [16,1040] overlaps pass-2 chunks 0/1 (staged to SBUF).
Pass 2: A = W1t^T h, Gt = Wgt^T h (fp8 DR, x16 weights); sigmoid-only
  activations (sg = sigmoid(A/16+a0) on scalar, w = (A+a0)/256 on vector,
  B = w*sg = silu/16 in fp8); Cp = (16*W2/2)^T B (true scale);
  out = h + (Cp + b2/2)*gate via vector stt + gpsimd add.
"""

import os
import ml_dtypes
import numpy as np

import concourse.bass as bass
import concourse.bacc as bacc
import concourse.tile as tile
import concourse.mybir as mybir
from concourse.bass_utils import run_bass_kernel_spmd

F32 = mybir.dt.float32
F32R = mybir.dt.float32r
BF16 = mybir.dt.bfloat16
FP8 = mybir.dt.float8e4
DR = mybir.MatmulPerfMode.DoubleRow
AF = mybir.ActivationFunctionType
OP = mybir.AluOpType
AX = mybir.AxisListType

NCORES = 8
N, D, H, HD, BIAS = 65536, 1024, 16, 64, 128
NS = N // NCORES            # 8192 rows per core
CH = 512                    # row-chunk
NCH = NS // CH              # 16 chunks
KT = D // 128               # 8 feature tiles
AUG = 1040                  # D + 2 stat cols + pad to %16
EPS = 1e-5
RES = 0.5

_CACHE = {}
LAST_RESULTS = None  # BassKernelResults from the most recent run (for test.py)


def _build(ncores=NCORES, variant="full"):
    nc = bacc.Bacc("TRN2", target_bir_lowering=False, debug=False,
                   num_devices=ncores)

    def din(name, shape, dt=BF16):
        return nc.dram_tensor(name, list(shape), dt, kind="ExternalInput").ap()

    # per-core tensors (pre-tiled: chunk c = rows [c*128,(c+1)*128))
    hT8t = din("hT8t", (NCH * 128, KT * CH), FP8)
    hTbt = din("hTbt", (NCH * 128, KT * CH))
    hN8t = din("hN8t", (NCH * 128, 4 * AUG), FP8)
    bT = din("bT", (BIAS, NS), FP8)
    rb = din("rb", (H, NS))               # r broadcast to 16 rows
    mncgb = din("mncgb", (H, NS))         # -256*cg_h * m_n
    # shared weights
    Wkp8t = din("Wkp8t", (128, KT * H), FP8)      # 256*Wkp tiled
    Wb = din("Wb", (BIAS, H), FP8)
    W1t8t = din("W1t8t", (128, KT * D), FP8)      # 16*W1[:D] tiled
    Wgt8t = din("Wgt8t", (128, KT * D), FP8)
    W2h8t = din("W2h8t", (128, KT * D), FP8)      # 16*0.5*W2 tiled
    Wvt = din("Wvt", (128, KT * D))               # bf16 tiled
    WAt = din("WAt", (128, KT * D))
    WGt = din("WGt", (128, KT * D))
    Wot = din("Wot", (128, KT * D))
    # small constants
    idn = din("idn", (128, 128), F32)
    idnb = din("idnb", (16, 16))
    cbv = din("cbv", (H, 1), F32)
    gb16 = din("gb16", (H, D), F32)
    bb16 = din("bb16", (H, D), F32)
    hcv = din("hcv", (128, KT), F32)
    a0c = din("a0c", (128, KT), F32)
    g0c = din("g0c", (128, KT), F32)
    b2v = din("b2v", (128, KT), F32)      # 0.5*b2

    outT = nc.dram_tensor("outT", [D, NS], F32, kind="ExternalOutput").ap()
    outC = nc.dram_tensor("outC", [128, KT], F32, kind="ExternalOutput").ap()

    with tile.TileContext(nc) as tc:
        with (
            tc.tile_pool(name="persist", bufs=1) as pp,
            tc.tile_pool(name="dram", bufs=1, space="DRAM") as dram,
        ):
            # ---- long-lived small tiles ----
            idn_s = pp.tile([128, 128], F32, tag="idn")
            nc.sync.dma_start(out=idn_s[:], in_=idn[:])
            idnb_s = pp.tile([16, 16], BF16, tag="idnb")
            nc.sync.dma_start(out=idnb_s[:], in_=idnb[:])
            cbv_s = pp.tile([H, 1], F32, tag="cbv")
            nc.sync.dma_start(out=cbv_s[:], in_=cbv[:])
            hcv_s = pp.tile([128, KT], F32, tag="hcv")
            nc.sync.dma_start(out=hcv_s[:], in_=hcv[:])
            a0c_s = pp.tile([128, KT], F32, tag="a0c")
            nc.sync.dma_start(out=a0c_s[:], in_=a0c[:])
            g0c_s = pp.tile([128, KT], F32, tag="g0c")
            nc.sync.dma_start(out=g0c_s[:], in_=g0c[:])
            b2v_s = pp.tile([128, KT], F32, tag="b2v")
            nc.sync.dma_start(out=b2v_s[:], in_=b2v[:])
            Wkp_s = pp.tile([128, KT, H], FP8, tag="Wkp")
            nc.sync.dma_start(out=Wkp_s[:, :, :], in_=Wkp8t[:])
            Wb_s = pp.tile([BIAS, H], FP8, tag="Wb")
            nc.sync.dma_start(out=Wb_s[:], in_=Wb[:])

            a0_s = pp.tile([128, KT], F32, tag="a0")
            a0d16_s = pp.tile([128, KT], F32, tag="a0d16")
            g0_s = pp.tile([128, KT], F32, tag="g0")

            # resident pass-2 stationary weights
            wres_cm = tc.tile_pool(name="p2w", bufs=1)
            wres = wres_cm.__enter__()
            W1t_s = wres.tile([128, KT, D], FP8, tag="W1t")
            Wgt_s = wres.tile([128, KT, D], FP8, tag="Wgt")
            W2h_s = wres.tile([128, KT, D], FP8, tag="W2h")

            # ======================= PASS 1 (single sweep) ==================
            p1s_cm = tc.tile_pool(name="p1state", bufs=1)
            p1s = p1s_cm.__enter__()
            rb_s = p1s.tile([H, NS], BF16, tag="rb")
            nc.sync.dma_start(out=rb_s[:], in_=rb[:])
            mncgb_s = p1s.tile([H, NS], BF16, tag="mncgb")
            nc.sync.dma_start(out=mncgb_s[:], in_=mncgb[:])
            Gacc = p1s.tile([H, AUG], F32, tag="Gacc")

            psG_cm = tc.tile_pool(name="p1psG", bufs=1, space="PSUM")
            psG = psG_cm.__enter__()
            G = psG.tile([H, AUG], F32, tag="G")
            with (
                tc.tile_pool(name="p1sb", bufs=2) as sbA,
                tc.tile_pool(name="p1sbn", bufs=3) as sbN,
                tc.tile_pool(name="p1sb1", bufs=2) as sbA1,
                tc.tile_pool(name="p1sbp", bufs=3) as sbP,
                tc.tile_pool(name="p1psL", bufs=3, space="PSUM") as psL,
                tc.tile_pool(name="p1ps1", bufs=2, space="PSUM") as ps1,
            ):
                hNcs, pTs = {}, {}

                def g_stage(cc):
                    pT = pTs.pop(cc)
                    hNc = hNcs.pop(cc)
                    tp = ps1.tile([128, 4 * H], BF16, tag="tp")
                    for j in range(4):
                        nc.tensor.transpose(
                            tp[:, j * H:(j + 1) * H],
                            pT[:, j * 128:(j + 1) * 128],
                            idnb_s[:])
                    p_nat = sbP.tile([128, 4, H], FP8, tag="p_nat")
                    nc.vector.tensor_copy(p_nat[:, :, :], tp[:])
                    first = (cc == 0)
                    last = (cc == NCH - 1)
                    for j in range(0, 4, 2):
                        for half in range(2):
                            nc.tensor.matmul(
                                G[:, half * CH:(half + 1) * CH],
                                p_nat[:, j:j + 2, :],
                                hNc[:, j:j + 2, half * CH:(half + 1) * CH],
                                start=(first and j == 0 and half == 0),
                                stop=(last and j == 2 and half == 1),
                                perf_mode=DR)
                        nc.tensor.matmul(
                            G[:, D:AUG],
                            p_nat[:, j:j + 2, :],
                            hNc[:, j:j + 2, D:AUG],
                            start=(first and j == 0),
                            stop=(last and j == 2),
                            perf_mode=DR)

                for c in range(NCH):
                    c0 = c * CH
                    r0 = c * 128
                    hTc = sbA.tile([128, KT, CH], FP8, tag="hTc")
                    for qq in range(4):
                        nc.sync.dma_start(
                            out=hTc[:, 2 * qq:2 * qq + 2, :],
                            in_=hT8t[r0:r0 + 128,
                                     qq * 2 * CH:(qq + 1) * 2 * CH])
                    hNc = sbN.tile([128, 4, AUG], FP8, tag="hNc")
                    hNcs[c] = hNc
                    for qq in range(4):
                        nc.sync.dma_start(
                            out=hNc[:, qq:qq + 1, :],
                            in_=hN8t[r0:r0 + 128,
                                     qq * AUG:(qq + 1) * AUG])
                    bTc = sbA1.tile([BIAS, CH], FP8, tag="bTc")
                    nc.sync.dma_start(out=bTc[:], in_=bT[:, c0:c0 + CH])
                    if c == 0:
                        nc.sync.dma_start(out=W1t_s[:, :, :], in_=W1t8t[:])
                    elif c == 1:
                        nc.sync.dma_start(out=Wgt_s[:, :, :], in_=Wgt8t[:])
                    elif c == 2:
                        nc.sync.dma_start(out=W2h_s[:, :, :], in_=W2h8t[:])

                    LL = psL.tile([48, CH], F32, tag="LL")
                    for k in range(0, KT, 2):
                        nc.tensor.matmul(LL[0:H, :], Wkp_s[:, k:k + 2, :],
                                         hTc[:, k:k + 2, :],
                                         start=(k == 0), stop=(k == KT - 2),
                                         perf_mode=DR)
                    nc.tensor.matmul(LL[32:48, :], Wb_s[:], bTc[:],
                                     start=True, stop=True)
                    t4 = sbA1.tile([H, CH], F32, tag="t4")
                    nc.vector.tensor_add(t4[:], LL[0:H, :],
                                         mncgb_s[:, c0:c0 + CH])
                    t5a = sbA1.tile([H, CH], F32, tag="t5a")
                    nc.vector.tensor_mul(t5a[:], t4[:], rb_s[:, c0:c0 + CH])
                    t5 = sbA1.tile([H, CH], F32, tag="t5")
                    nc.vector.scalar_tensor_tensor(
                        t5[:], t5a[:], 1.0 / 256.0, LL[32:48, :],
                        op0=OP.mult, op1=OP.add)
                    pT = sbP.tile([H, CH], BF16, tag="pT")
                    pTs[c] = pT
                    nc.scalar.activation(pT[:], t5[:], AF.Exp,
                                         bias=cbv_s[:, 0:1])
                    if c >= 2:
                        g_stage(c - 2)
                g_stage(NCH - 2)
                g_stage(NCH - 1)
                nc.vector.tensor_copy(Gacc[:], G[:])

            psG_cm.__exit__(None, None, None)
            arin = dram.tile([H, AUG], F32, tag="arin")
            arout = dram.tile([H, AUG], F32, tag="arout")
            nc.sync.dma_start(out=arin[:], in_=Gacc[:])
            if variant == "nocc":
                nc.sync.dma_start(out=arout[:], in_=arin[:])
            else:
                nc.gpsimd.collective_compute(
                    "AllReduce", OP.add,
                    replica_groups=[list(range(ncores))],
                    ins=[arin.opt()], outs=[arout.opt()])
            p1s_cm.__exit__(None, None, None)

            # ======================= PASS 2 =================================
            with (
                tc.tile_pool(name="p2sb", bufs=3) as sb3,
                tc.tile_pool(name="p2sbr", bufs=2) as sb3r,
                tc.tile_pool(name="p2bs", bufs=2) as sbBs,
                tc.tile_pool(name="p2sw", bufs=1) as sbSW,
                tc.tile_pool(name="p2st", bufs=3) as sb4,
                tc.tile_pool(name="p2ps", bufs=2, space="PSUM") as ps3,
                tc.tile_pool(name="p2cps", bufs=2, space="PSUM") as psC,
                tc.tile_pool(name="wstream", bufs=2) as ws,
            ):
                def load_htc2(c):
                    r0 = c * 128
                    t = sb3.tile([128, KT, CH], FP8, tag="hTc2")
                    for qq in range(4):
                        nc.sync.dma_start(
                            out=t[:, 2 * qq:2 * qq + 2, :],
                            in_=hT8t[r0:r0 + 128,
                                     qq * 2 * CH:(qq + 1) * 2 * CH])
                    tb = sb3r.tile([128, KT * CH], BF16, tag="hTb2")
                    for qq in range(4):
                        nc.sync.dma_start(
                            out=tb[:, qq * 2 * CH:(qq + 1) * 2 * CH],
                            in_=hTbt[r0:r0 + 128,
                                     qq * 2 * CH:(qq + 1) * 2 * CH])
                    return t, tb

                def a_group(hTc2, consume):
                    for m in range(KT):
                        A = ps3.tile([128, CH], F32, tag="A")
                        for k in range(0, KT, 2):
                            nc.tensor.matmul(
                                A[:], W1t_s[:, k:k + 2, m * 128:(m + 1) * 128],
                                hTc2[:, k:k + 2, :],
                                start=(k == 0), stop=(k == KT - 2),
                                perf_mode=DR)
                        consume(m, A[:])

                def gt_group(hTc2, consume):
                    for m in range(KT):
                        Gt = ps3.tile([128, CH], F32, tag="A")
                        for k in range(0, KT, 2):
                            nc.tensor.matmul(
                                Gt[:], Wgt_s[:, k:k + 2, m * 128:(m + 1) * 128],
                                hTc2[:, k:k + 2, :],
                                start=(k == 0), stop=(k == KT - 2),
                                perf_mode=DR)
                        consume(m, Gt[:])

                def cp_and_out(c, hTb2, Bs, gs):
                    c0 = c * CH
                    for m in range(KT):
                        Cp = psC.tile([128, CH], F32, tag="Cp")
                        for k in range(0, KT, 2):
                            nc.tensor.matmul(
                                Cp[:], W2h_s[:, k:k + 2, m * 128:(m + 1) * 128],
                                Bs[:, k:k + 2, :],
                                start=(k == 0), stop=(k == KT - 2),
                                perf_mode=DR)
                        t6 = sb4.tile([128, CH], F32, tag="t6")
                        nc.vector.scalar_tensor_tensor(
                            t6[:], Cp[:], b2v_s[:, m:m + 1],
                            gs[:, m * CH:(m + 1) * CH],
                            op0=OP.add, op1=OP.mult)
                        ot = sb4.tile([128, CH], F32, tag="ot")
                        nc.gpsimd.tensor_add(
                            ot[:], t6[:], hTb2[:, m * CH:(m + 1) * CH])
                        nc.sync.dma_start(
                            out=outT[m * 128:(m + 1) * 128, c0:c0 + CH],
                            in_=ot[:])

                def stage_copy_scalar(dst):
                    return lambda m, ps: nc.scalar.copy(
                        dst[:, m * CH:(m + 1) * CH], ps)

                def stage_copy_vector(dst):
                    return lambda m, ps: nc.vector.tensor_copy(
                        dst[:, m * CH:(m + 1) * CH], ps)

                def silu_into(Bs, sg, w):
                    def ff(m, src):
                        nc.scalar.activation(
                            sg[:, m * CH:(m + 1) * CH], src,
                            AF.Sigmoid, bias=a0_s[:, m:m + 1], scale=1.0 / 16.0)
                        nc.vector.tensor_scalar(
                            out=w[:, m * CH:(m + 1) * CH], in0=src,
                            scalar1=1.0 / 256.0, scalar2=a0d16_s[:, m:m + 1],
                            op0=OP.mult, op1=OP.add)
                        nc.vector.tensor_mul(
                            Bs[:, m, :], w[:, m * CH:(m + 1) * CH],
                            sg[:, m * CH:(m + 1) * CH])
                    return ff

                def sigm_into(gs):
                    return lambda m, src: nc.scalar.activation(
                        gs[:, m * CH:(m + 1) * CH], src,
                        AF.Sigmoid, bias=g0_s[:, m:m + 1], scale=1.0 / 16.0)

                # -- stage chunks 0,1 (fills the AllReduce window) --
                with tc.tile_pool(name="p2stage", bufs=1) as stg:
                    ht0, htb0 = load_htc2(0)
                    bsb0 = stg.tile([128, KT * CH], BF16, tag="bsb0")
                    a_group(ht0, stage_copy_scalar(bsb0))
                    gsb0 = stg.tile([128, KT * CH], BF16, tag="gsb0")
                    gt_group(ht0, stage_copy_vector(gsb0))
                    ht1, htb1 = load_htc2(1)
                    bsb1 = stg.tile([128, KT * CH], BF16, tag="bsb1")
                    a_group(ht1, stage_copy_scalar(bsb1))
                    gsb1 = stg.tile([128, KT * CH], BF16, tag="gsb1")
                    gt_group(ht1, stage_copy_vector(gsb1))

                    # -- post-collective block --
                    with (
                        tc.tile_pool(name="postsb", bufs=1) as psb,
                        tc.tile_pool(name="postps", bufs=1, space="PSUM") as ps2,
                    ):
                        gb16_s = psb.tile([H, D], F32, tag="gb16")
                        nc.sync.dma_start(out=gb16_s[:], in_=gb16[:])
                        bb16_s = psb.tile([H, D], F32, tag="bb16")
                        nc.sync.dma_start(out=bb16_s[:], in_=bb16[:])
                        Wv_s = ws.tile([128, KT * D], BF16, tag="wstream")
                        nc.sync.dma_start(out=Wv_s[:], in_=Wvt[:])
                        WA_s = ws.tile([128, KT * D], BF16, tag="wstream")
                        nc.sync.dma_start(out=WA_s[:], in_=WAt[:])
                        Gar = psb.tile([H, AUG], F32, tag="Gar")
                        nc.sync.dma_start(out=Gar[:], in_=arout[:])
                        PRMc = psb.tile([H, 1], F32, tag="PRMc")
                        nc.vector.tensor_scalar_mul(PRMc[:], Gar[:, D:D + 1],
                                                    1.0 / 32.0)
                        sr = psb.tile([H, 1], F32, tag="sr")
                        nc.vector.reciprocal_approx_fast(
                            out=sr[:], in_=Gar[:, D + 1:D + 2])
                        Gn = psb.tile([H, D], F32, tag="Gn")
                        nc.vector.tensor_scalar_sub(Gn[:], Gar[:, 0:D],
                                                    PRMc[:, 0:1])
                        nc.vector.tensor_mul(Gn[:], Gn[:], gb16_s[:])
                        nc.vector.scalar_tensor_tensor(
                            Gn[:], Gn[:], sr[:, 0:1], bb16_s[:],
                            op0=OP.mult, op1=OP.add)
                        tpg = ps2.tile([128, KT * H], F32, tag="post1")
                        for m in range(KT):
                            nc.tensor.transpose(
                                tpg[:, m * H:(m + 1) * H],
                                Gn[:, m * 128:(m + 1) * 128],
                                idn_s[0:16, 0:16])
                        GnT = psb.tile([128, KT * H], BF16, tag="GnT")
                        nc.vector.tensor_copy(GnT[:], tpg[:])

                        OCp = ps2.tile([128, KT * H], F32, tag="post1")
                        for m in range(KT):
                            for k in range(KT):
                                nc.tensor.matmul(
                                    OCp[:, m * H:(m + 1) * H],
                                    Wv_s[:, k * D + m * 128:k * D + (m + 1) * 128],
                                    GnT[:, k * H:(k + 1) * H],
                                    start=(k == 0), stop=(k == KT - 1))
                        ocv = psb.tile([128, KT], BF16, tag="ocv")
                        for m in range(KT):
                            if m % 2 == 0:
                                nc.vector.tensor_copy(
                                    ocv[0:64, m:m + 1],
                                    OCp[0:64, m * H + 2 * m:m * H + 2 * m + 1])
                                nc.vector.tensor_copy(
                                    ocv[64:128, m:m + 1],
                                    OCp[64:128, m * H + 2 * m + 1:m * H + 2 * m + 2])
                            else:
                                nc.scalar.copy(
                                    ocv[0:64, m:m + 1],
                                    OCp[0:64, m * H + 2 * m:m * H + 2 * m + 1])
                                nc.scalar.copy(
                                    ocv[64:128, m:m + 1],
                                    OCp[64:128, m * H + 2 * m + 1:m * H + 2 * m + 2])

                        a0p = ps2.tile([128, KT], F32, tag="smv")
                        for m in range(KT):
                            for k in range(KT):
                                nc.tensor.matmul(
                                    a0p[:, m:m + 1],
                                    WA_s[:, k * D + m * 128:k * D + (m + 1) * 128],
                                    ocv[:, k:k + 1],
                                    start=(k == 0), stop=(k == KT - 1))
                        nc.vector.scalar_tensor_tensor(
                            a0_s[:], a0p[:], RES, a0c_s[:],
                            op0=OP.mult, op1=OP.add)
                        nc.vector.tensor_scalar_mul(
                            a0d16_s[:], a0_s[:], 1.0 / 16.0)

                        WG_s = ws.tile([128, KT * D], BF16, tag="wstream")
                        nc.sync.dma_start(out=WG_s[:], in_=WGt[:])
                        g0p = ps2.tile([128, KT], F32, tag="smv")
                        for m in range(KT):
                            for k in range(KT):
                                nc.tensor.matmul(
                                    g0p[:, m:m + 1],
                                    WG_s[:, k * D + m * 128:k * D + (m + 1) * 128],
                                    ocv[:, k:k + 1],
                                    start=(k == 0), stop=(k == KT - 1))
                        nc.vector.scalar_tensor_tensor(
                            g0_s[:], g0p[:], RES, g0c_s[:],
                            op0=OP.mult, op1=OP.add)

                    # -- consume staged chunks --
                    for c, (htb, bsb, gsb) in ((0, (htb0, bsb0, gsb0)),
                                               (1, (htb1, bsb1, gsb1))):
                        Bs = sbBs.tile([128, KT, CH], FP8, tag="Bs")
                        sg = sbSW.tile([128, KT * CH], BF16, tag="sg")
                        w = sbSW.tile([128, KT * CH], BF16, tag="w")
                        fb = silu_into(Bs, sg, w)
                        for m in range(KT):
                            fb(m, bsb[:, m * CH:(m + 1) * CH])
                        gs = sbBs.tile([128, KT * CH], BF16, tag="gs")
                        fg = sigm_into(gs)
                        for m in range(KT):
                            fg(m, gsb[:, m * CH:(m + 1) * CH])
                        cp_and_out(c, htb, Bs, gs)

                    # h_c_new for the center row (off critical path)
                    Wo_s = ws.tile([128, KT * D], BF16, tag="wstream")
                    nc.sync.dma_start(out=Wo_s[:], in_=Wot[:])
                    hcp = psC.tile([128, KT], F32, tag="smv")
                    for m in range(KT):
                        for k in range(KT):
                            nc.tensor.matmul(
                                hcp[:, m:m + 1],
                                Wo_s[:, k * D + m * 128:k * D + (m + 1) * 128],
                                ocv[:, k:k + 1],
                                start=(k == 0), stop=(k == KT - 1))
                    hcn_sb = sbSW.tile([128, KT], F32, tag="hcn")
                    nc.vector.scalar_tensor_tensor(
                        hcn_sb[:], hcp[:], RES, hcv_s[:],
                        op0=OP.mult, op1=OP.add)
                    nc.sync.dma_start(out=outC[:], in_=hcn_sb[:])


                # -- remaining chunks: direct path, cp lags one chunk --
                pend = None
                for c in range(2, NCH):
                    htc, htb = load_htc2(c)
                    Bs = sbBs.tile([128, KT, CH], FP8, tag="Bs")
                    sg = sbSW.tile([128, KT * CH], BF16, tag="sg")
                    w = sbSW.tile([128, KT * CH], BF16, tag="w")
                    a_group(htc, silu_into(Bs, sg, w))
                    gs = sbBs.tile([128, KT * CH], BF16, tag="gs")
                    gt_group(htc, sigm_into(gs))
                    if pend is not None:
                        cp_and_out(*pend)
                    pend = (c, htb, Bs, gs)
                cp_and_out(*pend)

            wres_cm.__exit__(None, None, None)
    nc.compile()
    return nc


def _get_nc():
    if "nc" not in _CACHE:
        _CACHE["nc"] = _build()
    return _CACHE["nc"]


def _tile_rows(x, nblk, blk):
    """Feature-major (KT*128, nblk*blk) -> (nblk*128, KT*blk): chunk c of
    the column dim becomes rows [c*128,(c+1)*128) with KT feature blocks
    concatenated along columns."""
    kt = x.shape[0] // 128
    a = x.reshape(kt, 128, nblk, blk)
    return np.ascontiguousarray(
        a.transpose(2, 1, 0, 3).reshape(nblk * 128, kt * blk))


def kernel(h, center_idx, rbf_ic, seqsep_ic, nbr_idx, local_bias,
           gamma_c, beta_c, gamma_a, beta_a,
           Wq, Wk, Wv, Wo, Wb, W1, b1, W2, b2, Wg, bg):
    global LAST_RESULTS
    f = np.float32
    bf = ml_dtypes.bfloat16
    f8 = ml_dtypes.float8_e4m3fn
    h = np.asarray(h, f)
    c = int(center_idx)
    rbf_ic = np.asarray(rbf_ic, f)
    seqsep_ic = np.asarray(seqsep_ic, f)
    nbr_idx = np.asarray(nbr_idx)
    local_bias = np.asarray(local_bias, f)
    gamma_c = np.asarray(gamma_c, np.float64)
    beta_c = np.asarray(beta_c, np.float64)
    gamma_a = np.asarray(gamma_a, np.float64)
    beta_a = np.asarray(beta_a, np.float64)
    Wq = np.asarray(Wq, f); Wk = np.asarray(Wk, f); Wv = np.asarray(Wv, f)
    Wo = np.asarray(Wo, f); Wb = np.asarray(Wb, f)
    W1 = np.asarray(W1, f); b1 = np.asarray(b1, f)
    W2 = np.asarray(W2, f); b2 = np.asarray(b2, f)
    Wg = np.asarray(Wg, f); bg = np.asarray(bg, f)

    # ---- host algebra ----
    hc = h[c].astype(np.float64)
    hcl = (hc - hc.mean()) / np.sqrt(hc.var() + EPS) * gamma_c + beta_c
    q = (hcl @ Wq.astype(np.float64)).reshape(H, HD)
    Qm = np.zeros((D, H), np.float64)
    for hh in range(H):
        Qm[hh * HD:(hh + 1) * HD, hh] = q[hh] / np.sqrt(HD)
    Wk1 = Wk.astype(np.float64) @ Qm                    # (D, 16)
    Wkp8 = (256.0 * Wk1 * gamma_a[:, None]).astype(f8)
    ncg256 = -256.0 * (Wk1 * gamma_a[:, None]).sum(0)   # (16,) f64
    cbv = (Wk1 * beta_a[:, None]).sum(0).astype(f).reshape(H, 1)

    # LN stats on host
    m64 = h.mean(1, dtype=np.float64)                   # (N,)
    v64 = np.square(h.astype(np.float64)).mean(1) - m64 * m64
    sd64 = np.sqrt(v64 + EPS)
    r64 = 1.0 / sd64

    full_bias = np.zeros((N, local_bias.shape[1]), f)
    full_bias[nbr_idx] = local_bias
    bias_featT = np.ascontiguousarray(
        np.concatenate([rbf_ic, seqsep_ic, full_bias], axis=1).T)  # (128, N)

    # augmented, r-premultiplied h for the G matmul: [h | 32m | sd | pad]
    haug = np.zeros((N, AUG), f)
    haug[:, :D] = h * r64[:, None].astype(f)
    haug[:, D] = (32.0 * m64 * r64).astype(f)
    haug[:, D + 1] = 1.0                                # r*sd
    haug8 = haug.astype(f8)

    hT_full = np.ascontiguousarray(h.T)

    W1b = W1[D:]
    Wgb = Wg[D:]
    WA = (Wo.astype(np.float64) @ W1b.astype(np.float64)).astype(f)
    WG = (Wo.astype(np.float64) @ Wgb.astype(np.float64)).astype(f)
    a0c = (h[c].astype(np.float64) @ W1b.astype(np.float64) + b1).astype(f)
    g0c = (h[c].astype(np.float64) @ Wgb.astype(np.float64) + bg).astype(f)

    def wtile(x, dt):
        return np.ascontiguousarray(
            np.asarray(x, f).reshape(KT, 128, D).transpose(1, 0, 2)
            .reshape(128, KT * D)).astype(dt)

    gamma_a32 = gamma_a.astype(f)
    beta_a32 = beta_a.astype(f)
    Wb_pad = Wb
    shared = {
        "Wkp8t": np.ascontiguousarray(
            Wkp8.reshape(KT, 128, H).transpose(1, 0, 2).reshape(128, KT * H)),
        "Wb": Wb.astype(f8),
        "W1t8t": wtile(16.0 * W1[:D], f8),
        "Wgt8t": wtile(16.0 * Wg[:D], f8),
        "W2h8t": wtile(16.0 * RES * W2, f8),
        "Wvt": wtile(Wv, bf), "Wot": wtile(Wo, bf),
        "WAt": wtile(WA, bf), "WGt": wtile(WG, bf),
        "idn": np.eye(128, dtype=f),
        "idnb": np.eye(16, dtype=bf),
        "cbv": cbv,
        "gb16": np.ascontiguousarray(np.broadcast_to(gamma_a32, (H, D))),
        "bb16": np.ascontiguousarray(np.broadcast_to(beta_a32, (H, D))),
        "hcv": np.ascontiguousarray(h[c].reshape(KT, 128).T),
        "a0c": np.ascontiguousarray(a0c.reshape(KT, 128).T),
        "g0c": np.ascontiguousarray(g0c.reshape(KT, 128).T),
        "b2v": np.ascontiguousarray((RES * b2).reshape(KT, 128).T),
    }
    in_maps = []
    for i in range(NCORES):
        r0 = i * NS
        mm = dict(shared)
        hTs = hT_full[:, r0:r0 + NS]
        mm["hT8t"] = _tile_rows(hTs, NCH, CH).astype(f8)
        mm["hTbt"] = _tile_rows(hTs, NCH, CH).astype(bf)
        ha = haug8[r0:r0 + NS]            # (NS, AUG) fp8
        mm["hN8t"] = np.ascontiguousarray(
            ha.reshape(NCH, 4, 128, AUG).transpose(0, 2, 1, 3)
            .reshape(NCH * 128, 4 * AUG))
        mm["bT"] = np.ascontiguousarray(bias_featT[:, r0:r0 + NS]).astype(f8)
        rr = r64[r0:r0 + NS].astype(bf)
        mm["rb"] = np.ascontiguousarray(np.broadcast_to(rr, (H, NS)))
        mm["mncgb"] = (ncg256[:, None] * m64[None, r0:r0 + NS]).astype(bf)
        in_maps.append(mm)

    nc = _get_nc()
    trace = bool(int(os.environ.get("KERNEL_TRACE", "0")))
    res = run_bass_kernel_spmd(nc, in_maps, core_ids=list(range(NCORES)),
                               trace=trace)
    LAST_RESULTS = res

    out = np.empty((N, D), f)
    for i in range(NCORES):
        out[i * NS:(i + 1) * NS] = res.results[i]["outT"].T
    hcn = res.results[0]["outC"].T.reshape(D)           # [m,p] -> flat
    out[c] = hcn
    return out


# revision 28
# speedup vs baseline: 1.0892x; 1.0892x over previous
"""Trainium2 Bass kernel for CenterGeoAttention (N=65536, D=1024, H=16).

Row-shard N across 8 cores; fp8 DoubleRow matmuls; host-folded algebra.

Host precomputes (all cheap, O(N*D) or one D*D matmul):
  - LN stats r=1/sd, m per node; logits fold Wkp=(Wk@Qdiag)*gamma*256 (fp8),
    mncgb[h,n] = -256*cg_h*m_n, rb[h,n]=r_n (bf16, 16 rows).
  - h_aug[n] = r_n*[h | 32m | sd | 0pad] (fp8, 1040 cols) so ONE G matmul
    accumulates [G | 32*PRM | S] for the softmax partial sums.
  - a0c/g0c = W1b^T h_c + b1 etc; WA=Wo@W1b, WG=Wo@Wgb shorten the
    post-collective chain to Gn -> ocv -> a0/g0.
  - All streamed tensors pre-tiled so each chunk is ONE DMA with >=4KB
    per-partition lines (fp8 512B lines otherwise shatter into 45k
    descriptors).

Device, pass 1 (single sweep, 16 chunks): L = Wkp^T h (fp8 DR) + bias-proj
  L2; p = exp(r*(L-cg*m)/256 + L2 + cb); G += p^T h_aug (fp8 DR).
AllReduce [16,1040] overlaps pass-2 chunks 0/1 (staged to SBUF).
Pass 2: A = W1t^T h, Gt = Wgt^T h (fp8 DR, x16 weights); sigmoid-only
  activations (sg = sigmoid(A/16+a0) on scalar, w = (A+a0)/256 on vector,
  B = w*sg = silu/16 in fp8); Cp = (16*W2/2)^T B (true scale);
  out = h + (Cp + b2/2)*gate via vector stt + gpsimd add.
"""

import os
import ml_dtypes
import numpy as np

import concourse.bass as bass
import concourse.bacc as bacc
import concourse.tile as tile
import concourse.mybir as mybir
from concourse.bass_utils import run_bass_kernel_spmd

F32 = mybir.dt.float32
F32R = mybir.dt.float32r
BF16 = mybir.dt.bfloat16
FP8 = mybir.dt.float8e4
DR = mybir.MatmulPerfMode.DoubleRow
AF = mybir.ActivationFunctionType
OP = mybir.AluOpType
AX = mybir.AxisListType

NCORES = 8
N, D, H, HD, BIAS = 65536, 1024, 16, 64, 128
NS = N // NCORES            # 8192 rows per core
CH = 512                    # row-chunk
NCH = NS // CH              # 16 chunks
KT = D // 128               # 8 feature tiles
AUG = 1040                  # D + 2 stat cols + pad to %16
EPS = 1e-5
RES = 0.5

_CACHE = {}
LAST_RESULTS = None  # BassKernelResults from the most recent run (for test.py)


def _build(ncores=NCORES, variant="full"):
    nc = bacc.Bacc("TRN2", target_bir_lowering=False, debug=False,
                   num_devices=ncores)

    def din(name, shape, dt=BF16):
        return nc.dram_tensor(name, list(shape), dt, kind="ExternalInput").ap()

    # per-core tensors (pre-tiled: chunk c = rows [c*128,(c+1)*128))
    hT8t = din("hT8t", (NCH * 128, KT * CH), FP8)
    hTbt = din("hTbt", (NCH * 128, KT * CH))
    hN8t = din("hN8t", (NCH * 128, 4 * AUG), FP8)
    bT = din("bT", (BIAS, NS), FP8)
    rb = din("rb", (H, NS))               # r broadcast to 16 rows
    mncgb = din("mncgb", (H, NS))         # -256*cg_h * m_n
    # shared weights
    Wkp8t = din("Wkp8t", (128, KT * H), FP8)      # 256*Wkp tiled
    Wb = din("Wb", (BIAS, H), FP8)
    W1t8t = din("W1t8t", (128, KT * D), FP8)      # 16*W1[:D] tiled
    Wgt8t = din("Wgt8t", (128, KT * D), FP8)
    W2h8t = din("W2h8t", (128, KT * D), FP8)      # 16*0.5*W2 tiled
    Wvt = din("Wvt", (128, KT * D))               # bf16 tiled
    WAt = din("WAt", (128, KT * D))
    WGt = din("WGt", (128, KT * D))
    Wot = din("Wot", (128, KT * D))
    # small constants
    idn = din("idn", (128, 128), F32)
    idnb = din("idnb", (16, 16))
    cbv = din("cbv", (H, 1), F32)
    gb16 = din("gb16", (H, D), F32)
    bb16 = din("bb16", (H, D), F32)
    hcv = din("hcv", (128, KT), F32)
    a0c = din("a0c", (128, KT), F32)
    g0c = din("g0c", (128, KT), F32)
    b2v = din("b2v", (128, KT), F32)      # 0.5*b2

    outT = nc.dram_tensor("outT", [D, NS], F32, kind="ExternalOutput").ap()
    outC = nc.dram_tensor("outC", [128, KT], F32, kind="ExternalOutput").ap()

    with tile.TileContext(nc) as tc:
        with (
            tc.tile_pool(name="persist", bufs=1) as pp,
            tc.tile_pool(name="dram", bufs=1, space="DRAM") as dram,
        ):
            # ---- long-lived small tiles ----
            idn_s = pp.tile([128, 128], F32, tag="idn")
            nc.sync.dma_start(out=idn_s[:], in_=idn[:])
            idnb_s = pp.tile([16, 16], BF16, tag="idnb")
            nc.sync.dma_start(out=idnb_s[:], in_=idnb[:])
            cbv_s = pp.tile([H, 1], F32, tag="cbv")
            nc.sync.dma_start(out=cbv_s[:], in_=cbv[:])
            hcv_s = pp.tile([128, KT], F32, tag="hcv")
            nc.sync.dma_start(out=hcv_s[:], in_=hcv[:])
            a0c_s = pp.tile([128, KT], F32, tag="a0c")
            nc.sync.dma_start(out=a0c_s[:], in_=a0c[:])
            g0c_s = pp.tile([128, KT], F32, tag="g0c")
            nc.sync.dma_start(out=g0c_s[:], in_=g0c[:])
            b2v_s = pp.tile([128, KT], F32, tag="b2v")
            nc.sync.dma_start(out=b2v_s[:], in_=b2v[:])
            Wkp_s = pp.tile([128, KT, H], FP8, tag="Wkp")
            nc.sync.dma_start(out=Wkp_s[:, :, :], in_=Wkp8t[:])
            Wb_s = pp.tile([BIAS, H], FP8, tag="Wb")
            nc.sync.dma_start(out=Wb_s[:], in_=Wb[:])

            a0_s = pp.tile([128, KT], F32, tag="a0")
            a0d16_s = pp.tile([128, KT], F32, tag="a0d16")
            g0_s = pp.tile([128, KT], F32, tag="g0")

            # resident pass-2 stationary weights
            wres_cm = tc.tile_pool(name="p2w", bufs=1)
            wres = wres_cm.__enter__()
            W1t_s = wres.tile([128, KT, D], FP8, tag="W1t")
            Wgt_s = wres.tile([128, KT, D], FP8, tag="Wgt")
            W2h_s = wres.tile([128, KT, D], FP8, tag="W2h")

            # ======================= PASS 1 (single sweep) ==================
            p1s_cm = tc.tile_pool(name="p1state", bufs=1)
            p1s = p1s_cm.__enter__()
            rb_s = p1s.tile([H, NS], BF16, tag="rb")
            nc.sync.dma_start(out=rb_s[:], in_=rb[:])
            mncgb_s = p1s.tile([H, NS], BF16, tag="mncgb")
            nc.sync.dma_start(out=mncgb_s[:], in_=mncgb[:])
            Gacc = p1s.tile([H, AUG], F32, tag="Gacc")

            psG_cm = tc.tile_pool(name="p1psG", bufs=1, space="PSUM")
            psG = psG_cm.__enter__()
            G = psG.tile([H, AUG], F32, tag="G")
            with (
                tc.tile_pool(name="p1sb", bufs=2) as sbA,
                tc.tile_pool(name="p1sbn", bufs=3) as sbN,
                tc.tile_pool(name="p1sb1", bufs=2) as sbA1,
                tc.tile_pool(name="p1sbp", bufs=3) as sbP,
                tc.tile_pool(name="p1psL", bufs=3, space="PSUM") as psL,
                tc.tile_pool(name="p1ps1", bufs=2, space="PSUM") as ps1,
            ):
                hNcs, pTs = {}, {}

                def g_stage(cc):
                    pT = pTs.pop(cc)
                    hNc = hNcs.pop(cc)
                    tp = ps1.tile([128, 4 * H], BF16, tag="tp")
                    for j in range(4):
                        nc.tensor.transpose(
                            tp[:, j * H:(j + 1) * H],
                            pT[:, j * 128:(j + 1) * 128],
                            idnb_s[:])
                    p_nat = sbP.tile([128, 4, H], FP8, tag="p_nat")
                    nc.vector.tensor_copy(p_nat[:, :, :], tp[:])
                    first = (cc == 0)
                    last = (cc == NCH - 1)
                    for j in range(0, 4, 2):
                        for half in range(2):
                            nc.tensor.matmul(
                                G[:, half * CH:(half + 1) * CH],
                                p_nat[:, j:j + 2, :],
                                hNc[:, j:j + 2, half * CH:(half + 1) * CH],
                                start=(first and j == 0 and half == 0),
                                stop=(last and j == 2 and half == 1),
                                perf_mode=DR)
                        nc.tensor.matmul(
                            G[:, D:AUG],
                            p_nat[:, j:j + 2, :],
                            hNc[:, j:j + 2, D:AUG],
                            start=(first and j == 0),
                            stop=(last and j == 2),
                            perf_mode=DR)

                for c in range(NCH):
                    c0 = c * CH
                    r0 = c * 128
                    hTc = sbA.tile([128, KT, CH], FP8, tag="hTc")
                    for qq in range(2):
                        nc.sync.dma_start(
                            out=hTc[:, 4 * qq:4 * qq + 4, :],
                            in_=hT8t[r0:r0 + 128,
                                     qq * 4 * CH:(qq + 1) * 4 * CH])
                    hNc = sbN.tile([128, 4, AUG], FP8, tag="hNc")
                    hNcs[c] = hNc
                    for qq in range(2):
                        nc.sync.dma_start(
                            out=hNc[:, 2 * qq:2 * qq + 2, :],
                            in_=hN8t[r0:r0 + 128,
                                     qq * 2 * AUG:(qq + 1) * 2 * AUG])
                    bTc = sbA1.tile([BIAS, CH], FP8, tag="bTc")
                    nc.sync.dma_start(out=bTc[:], in_=bT[:, c0:c0 + CH])
                    if c == 0:
                        nc.sync.dma_start(out=W1t_s[:, :, :], in_=W1t8t[:])
                    elif c == 1:
                        nc.sync.dma_start(out=Wgt_s[:, :, :], in_=Wgt8t[:])
                    elif c == 2:
                        nc.sync.dma_start(out=W2h_s[:, :, :], in_=W2h8t[:])

                    LL = psL.tile([48, CH], F32, tag="LL")
                    for k in range(0, KT, 2):
                        nc.tensor.matmul(LL[0:H, :], Wkp_s[:, k:k + 2, :],
                                         hTc[:, k:k + 2, :],
                                         start=(k == 0), stop=(k == KT - 2),
                                         perf_mode=DR)
                    nc.tensor.matmul(LL[32:48, :], Wb_s[:], bTc[:],
                                     start=True, stop=True)
                    t4 = sbA1.tile([H, CH], F32, tag="t4")
                    nc.vector.tensor_add(t4[:], LL[0:H, :],
                                         mncgb_s[:, c0:c0 + CH])
                    t5a = sbA1.tile([H, CH], F32, tag="t5a")
                    nc.vector.tensor_mul(t5a[:], t4[:], rb_s[:, c0:c0 + CH])
                    t5 = sbA1.tile([H, CH], F32, tag="t5")
                    nc.vector.scalar_tensor_tensor(
                        t5[:], t5a[:], 1.0 / 256.0, LL[32:48, :],
                        op0=OP.mult, op1=OP.add)
                    pT = sbP.tile([H, CH], BF16, tag="pT")
                    pTs[c] = pT
                    nc.scalar.activation(pT[:], t5[:], AF.Exp,
                                         bias=cbv_s[:, 0:1])
                    if c >= 2:
                        g_stage(c - 2)
                g_stage(NCH - 2)
                g_stage(NCH - 1)
                nc.vector.tensor_copy(Gacc[:], G[:])

            psG_cm.__exit__(None, None, None)
            arin = dram.tile([H, AUG], F32, tag="arin")
            arout = dram.tile([H, AUG], F32, tag="arout")
            nc.sync.dma_start(out=arin[:], in_=Gacc[:])
            if variant == "nocc":
                nc.sync.dma_start(out=arout[:], in_=arin[:])
            else:
                nc.gpsimd.collective_compute(
                    "AllReduce", OP.add,
                    replica_groups=[list(range(ncores))],
                    ins=[arin.opt()], outs=[arout.opt()])
            p1s_cm.__exit__(None, None, None)

            # ======================= PASS 2 =================================
            with (
                tc.tile_pool(name="p2sb", bufs=3) as sb3,
                tc.tile_pool(name="p2sbr", bufs=2) as sb3r,
                tc.tile_pool(name="p2bs", bufs=2) as sbBs,
                tc.tile_pool(name="p2sw", bufs=1) as sbSW,
                tc.tile_pool(name="p2st", bufs=3) as sb4,
                tc.tile_pool(name="p2ps", bufs=2, space="PSUM") as ps3,
                tc.tile_pool(name="p2cps", bufs=2, space="PSUM") as psC,
                tc.tile_pool(name="wstream", bufs=2) as ws,
            ):
                def load_htc2(c):
                    r0 = c * 128
                    t = sb3.tile([128, KT, CH], FP8, tag="hTc2")
                    for qq in range(2):
                        nc.sync.dma_start(
                            out=t[:, 4 * qq:4 * qq + 4, :],
                            in_=hT8t[r0:r0 + 128,
                                     qq * 4 * CH:(qq + 1) * 4 * CH])
                    tb = sb3r.tile([128, KT * CH], BF16, tag="hTb2")
                    for qq in range(2):
                        nc.sync.dma_start(
                            out=tb[:, qq * 4 * CH:(qq + 1) * 4 * CH],
                            in_=hTbt[r0:r0 + 128,
                                     qq * 4 * CH:(qq + 1) * 4 * CH])
                    return t, tb

                def a_group(hTc2, consume):
                    for m in range(KT):
                        A = ps3.tile([128, CH], F32, tag="A")
                        for k in range(0, KT, 2):
                            nc.tensor.matmul(
                                A[:], W1t_s[:, k:k + 2, m * 128:(m + 1) * 128],
                                hTc2[:, k:k + 2, :],
                                start=(k == 0), stop=(k == KT - 2),
                                perf_mode=DR)
                        consume(m, A[:])

                def gt_group(hTc2, consume):
                    for m in range(KT):
                        Gt = ps3.tile([128, CH], F32, tag="Gt")
                        for k in range(0, KT, 2):
                            nc.tensor.matmul(
                                Gt[:], Wgt_s[:, k:k + 2, m * 128:(m + 1) * 128],
                                hTc2[:, k:k + 2, :],
                                start=(k == 0), stop=(k == KT - 2),
                                perf_mode=DR)
                        consume(m, Gt[:])

                def cp_and_out(c, hTb2, Bs, gs):
                    c0 = c * CH
                    for m in range(KT):
                        Cp = psC.tile([128, CH], F32, tag="Cp")
                        for k in range(0, KT, 2):
                            nc.tensor.matmul(
                                Cp[:], W2h_s[:, k:k + 2, m * 128:(m + 1) * 128],
                                Bs[:, k:k + 2, :],
                                start=(k == 0), stop=(k == KT - 2),
                                perf_mode=DR)
                        t6 = sb4.tile([128, CH], F32, tag="t6")
                        nc.vector.scalar_tensor_tensor(
                            t6[:], Cp[:], b2v_s[:, m:m + 1],
                            gs[:, m * CH:(m + 1) * CH],
                            op0=OP.add, op1=OP.mult)
                        ot = sb4.tile([128, CH], F32, tag="ot")
                        nc.gpsimd.tensor_add(
                            ot[:], t6[:], hTb2[:, m * CH:(m + 1) * CH])
                        nc.sync.dma_start(
                            out=outT[m * 128:(m + 1) * 128, c0:c0 + CH],
                            in_=ot[:])

                def stage_copy_scalar(dst):
                    return lambda m, ps: nc.scalar.copy(
                        dst[:, m * CH:(m + 1) * CH], ps)

                def stage_copy_vector(dst):
                    return lambda m, ps: nc.vector.tensor_copy(
                        dst[:, m * CH:(m + 1) * CH], ps)

                def silu_into(Bs, sg, w):
                    def ff(m, src):
                        nc.scalar.activation(
                            sg[:, m * CH:(m + 1) * CH], src,
                            AF.Sigmoid, bias=a0_s[:, m:m + 1], scale=1.0 / 16.0)
                        nc.vector.tensor_scalar(
                            out=w[:, m * CH:(m + 1) * CH], in0=src,
                            scalar1=1.0 / 256.0, scalar2=a0d16_s[:, m:m + 1],
                            op0=OP.mult, op1=OP.add)
                        nc.vector.tensor_mul(
                            Bs[:, m, :], w[:, m * CH:(m + 1) * CH],
                            sg[:, m * CH:(m + 1) * CH])
                    return ff

                def sigm_into(gs):
                    return lambda m, src: nc.scalar.activation(
                        gs[:, m * CH:(m + 1) * CH], src,
                        AF.Sigmoid, bias=g0_s[:, m:m + 1], scale=1.0 / 16.0)

                # -- stage chunks 0,1 (fills the AllReduce window) --
                with tc.tile_pool(name="p2stage", bufs=1) as stg:
                    ht0, htb0 = load_htc2(0)
                    bsb0 = stg.tile([128, KT * CH], BF16, tag="bsb0")
                    a_group(ht0, stage_copy_scalar(bsb0))
                    gsb0 = stg.tile([128, KT * CH], BF16, tag="gsb0")
                    gt_group(ht0, stage_copy_vector(gsb0))
                    ht1, htb1 = load_htc2(1)
                    bsb1 = stg.tile([128, KT * CH], BF16, tag="bsb1")
                    a_group(ht1, stage_copy_scalar(bsb1))
                    gsb1 = stg.tile([128, KT * CH], BF16, tag="gsb1")
                    gt_group(ht1, stage_copy_vector(gsb1))

                    # -- post-collective block --
                    with (
                        tc.tile_pool(name="postsb", bufs=1) as psb,
                        tc.tile_pool(name="postps", bufs=1, space="PSUM") as ps2,
                    ):
                        gb16_s = psb.tile([H, D], F32, tag="gb16")
                        nc.sync.dma_start(out=gb16_s[:], in_=gb16[:])
                        bb16_s = psb.tile([H, D], F32, tag="bb16")
                        nc.sync.dma_start(out=bb16_s[:], in_=bb16[:])
                        Wv_s = ws.tile([128, KT * D], BF16, tag="wstream")
                        nc.sync.dma_start(out=Wv_s[:], in_=Wvt[:])
                        WA_s = ws.tile([128, KT * D], BF16, tag="wstream")
                        nc.sync.dma_start(out=WA_s[:], in_=WAt[:])
                        Gar = psb.tile([H, AUG], F32, tag="Gar")
                        nc.sync.dma_start(out=Gar[:], in_=arout[:])
                        PRMc = psb.tile([H, 1], F32, tag="PRMc")
                        nc.vector.tensor_scalar_mul(PRMc[:], Gar[:, D:D + 1],
                                                    1.0 / 32.0)
                        sr = psb.tile([H, 1], F32, tag="sr")
                        nc.vector.reciprocal_approx_fast(
                            out=sr[:], in_=Gar[:, D + 1:D + 2])
                        Gn = psb.tile([H, D], F32, tag="Gn")
                        nc.vector.tensor_scalar_sub(Gn[:], Gar[:, 0:D],
                                                    PRMc[:, 0:1])
                        nc.vector.tensor_mul(Gn[:], Gn[:], gb16_s[:])
                        nc.vector.scalar_tensor_tensor(
                            Gn[:], Gn[:], sr[:, 0:1], bb16_s[:],
                            op0=OP.mult, op1=OP.add)
                        tpg = ps2.tile([128, KT * H], F32, tag="post1")
                        for m in range(KT):
                            nc.tensor.transpose(
                                tpg[:, m * H:(m + 1) * H],
                                Gn[:, m * 128:(m + 1) * 128],
                                idn_s[0:16, 0:16])
                        GnT = psb.tile([128, KT * H], BF16, tag="GnT")
                        nc.vector.tensor_copy(GnT[:], tpg[:])

                        OCp = ps2.tile([128, KT * H], F32, tag="post1")
                        for m in range(KT):
                            for k in range(KT):
                                nc.tensor.matmul(
                                    OCp[:, m * H:(m + 1) * H],
                                    Wv_s[:, k * D + m * 128:k * D + (m + 1) * 128],
                                    GnT[:, k * H:(k + 1) * H],
                                    start=(k == 0), stop=(k == KT - 1))
                        ocv = pp.tile([128, KT], BF16, tag="ocv")
                        for m in range(KT):
                            if m % 2 == 0:
                                nc.vector.tensor_copy(
                                    ocv[0:64, m:m + 1],
                                    OCp[0:64, m * H + 2 * m:m * H + 2 * m + 1])
                                nc.vector.tensor_copy(
                                    ocv[64:128, m:m + 1],
                                    OCp[64:128, m * H + 2 * m + 1:m * H + 2 * m + 2])
                            else:
                                nc.scalar.copy(
                                    ocv[0:64, m:m + 1],
                                    OCp[0:64, m * H + 2 * m:m * H + 2 * m + 1])
                                nc.scalar.copy(
                                    ocv[64:128, m:m + 1],
                                    OCp[64:128, m * H + 2 * m + 1:m * H + 2 * m + 2])

                        a0p = ps2.tile([128, KT], F32, tag="smv")
                        for m in range(KT):
                            for k in range(KT):
                                nc.tensor.matmul(
                                    a0p[:, m:m + 1],
                                    WA_s[:, k * D + m * 128:k * D + (m + 1) * 128],
                                    ocv[:, k:k + 1],
                                    start=(k == 0), stop=(k == KT - 1))
                        nc.vector.scalar_tensor_tensor(
                            a0_s[:], a0p[:], RES, a0c_s[:],
                            op0=OP.mult, op1=OP.add)
                        nc.vector.tensor_scalar_mul(
                            a0d16_s[:], a0_s[:], 1.0 / 16.0)

                        WG_s = ws.tile([128, KT * D], BF16, tag="wstream")
                        nc.sync.dma_start(out=WG_s[:], in_=WGt[:])
                        g0p = ps2.tile([128, KT], F32, tag="smv")
                        for m in range(KT):
                            for k in range(KT):
                                nc.tensor.matmul(
                                    g0p[:, m:m + 1],
                                    WG_s[:, k * D + m * 128:k * D + (m + 1) * 128],
                                    ocv[:, k:k + 1],
                                    start=(k == 0), stop=(k == KT - 1))
                        nc.vector.scalar_tensor_tensor(
                            g0_s[:], g0p[:], RES, g0c_s[:],
                            op0=OP.mult, op1=OP.add)

                    # -- consume staged chunks --
                    for c, (htb, bsb, gsb) in ((0, (htb0, bsb0, gsb0)),
                                               (1, (htb1, bsb1, gsb1))):
                        Bs = sbBs.tile([128, KT, CH], FP8, tag="Bs")
                        sg = sbSW.tile([128, KT * CH], BF16, tag="sg")
                        w = sbSW.tile([128, KT * CH], BF16, tag="w")
                        fb = silu_into(Bs, sg, w)
                        for m in range(KT):
                            fb(m, bsb[:, m * CH:(m + 1) * CH])
                        gs = sbBs.tile([128, KT * CH], BF16, tag="gs")
                        fg = sigm_into(gs)
                        for m in range(KT):
                            fg(m, gsb[:, m * CH:(m + 1) * CH])
                        cp_and_out(c, htb, Bs, gs)

                    # h_c_new for the center row (off critical path)
                    Wo_s = ws.tile([128, KT * D], BF16, tag="wstream")
                    nc.sync.dma_start(out=Wo_s[:], in_=Wot[:])
                    pshc_cm = tc.tile_pool(name="pshc", bufs=1, space="PSUM")
                    pshc = pshc_cm.__enter__()
                    hcp = pshc.tile([128, KT], F32, tag="smv")
                    for m in range(KT):
                        for k in range(KT):
                            nc.tensor.matmul(
                                hcp[:, m:m + 1],
                                Wo_s[:, k * D + m * 128:k * D + (m + 1) * 128],
                                ocv[:, k:k + 1],
                                start=(k == 0), stop=(k == KT - 1))
                    hcn_sb = sbSW.tile([128, KT], F32, tag="hcn")
                    nc.vector.scalar_tensor_tensor(
                        hcn_sb[:], hcp[:], RES, hcv_s[:],
                        op0=OP.mult, op1=OP.add)
                    nc.sync.dma_start(out=outC[:], in_=hcn_sb[:])
                    pshc_cm.__exit__(None, None, None)


                # -- remaining chunks: direct path, cp lags one chunk --
                pend = None
                for c in range(2, NCH):
                    htc, htb = load_htc2(c)
                    Bs = sbBs.tile([128, KT, CH], FP8, tag="Bs")
                    sg = sbSW.tile([128, KT * CH], BF16, tag="sg")
                    w = sbSW.tile([128, KT * CH], BF16, tag="w")
                    a_group(htc, silu_into(Bs, sg, w))
                    gs = sbBs.tile([128, KT * CH], BF16, tag="gs")
                    gt_group(htc, sigm_into(gs))
                    if pend is not None:
                        cp_and_out(*pend)
                    pend = (c, htb, Bs, gs)
                cp_and_out(*pend)

            wres_cm.__exit__(None, None, None)
    nc.compile()
    return nc


def _get_nc():
    if "nc" not in _CACHE:
        _CACHE["nc"] = _build()
    return _CACHE["nc"]


def _tile_rows(x, nblk, blk):
    """Feature-major (KT*128, nblk*blk) -> (nblk*128, KT*blk): chunk c of
    the column dim becomes rows [c*128,(c+1)*128) with KT feature blocks
    concatenated along columns."""
    kt = x.shape[0] // 128
    a = x.reshape(kt, 128, nblk, blk)
    return np.ascontiguousarray(
        a.transpose(2, 1, 0, 3).reshape(nblk * 128, kt * blk))


def kernel(h, center_idx, rbf_ic, seqsep_ic, nbr_idx, local_bias,
           gamma_c, beta_c, gamma_a, beta_a,
           Wq, Wk, Wv, Wo, Wb, W1, b1, W2, b2, Wg, bg):
    global LAST_RESULTS
    f = np.float32
    bf = ml_dtypes.bfloat16
    f8 = ml_dtypes.float8_e4m3fn
    h = np.asarray(h, f)
    c = int(center_idx)
    rbf_ic = np.asarray(rbf_ic, f)
    seqsep_ic = np.asarray(seqsep_ic, f)
    nbr_idx = np.asarray(nbr_idx)
    local_bias = np.asarray(local_bias, f)
    gamma_c = np.asarray(gamma_c, np.float64)
    beta_c = np.asarray(beta_c, np.float64)
    gamma_a = np.asarray(gamma_a, np.float64)
    beta_a = np.asarray(beta_a, np.float64)
    Wq = np.asarray(Wq, f); Wk = np.asarray(Wk, f); Wv = np.asarray(Wv, f)
    Wo = np.asarray(Wo, f); Wb = np.asarray(Wb, f)
    W1 = np.asarray(W1, f); b1 = np.asarray(b1, f)
    W2 = np.asarray(W2, f); b2 = np.asarray(b2, f)
    Wg = np.asarray(Wg, f); bg = np.asarray(bg, f)

    # ---- host algebra ----
    hc = h[c].astype(np.float64)
    hcl = (hc - hc.mean()) / np.sqrt(hc.var() + EPS) * gamma_c + beta_c
    q = (hcl @ Wq.astype(np.float64)).reshape(H, HD)
    Qm = np.zeros((D, H), np.float64)
    for hh in range(H):
        Qm[hh * HD:(hh + 1) * HD, hh] = q[hh] / np.sqrt(HD)
    Wk1 = Wk.astype(np.float64) @ Qm                    # (D, 16)
    Wkp8 = (256.0 * Wk1 * gamma_a[:, None]).astype(f8)
    ncg256 = -256.0 * (Wk1 * gamma_a[:, None]).sum(0)   # (16,) f64
    cbv = (Wk1 * beta_a[:, None]).sum(0).astype(f).reshape(H, 1)

    # LN stats on host
    m64 = h.mean(1, dtype=np.float64)                   # (N,)
    v64 = np.square(h.astype(np.float64)).mean(1) - m64 * m64
    sd64 = np.sqrt(v64 + EPS)
    r64 = 1.0 / sd64

    full_bias = np.zeros((N, local_bias.shape[1]), f)
    full_bias[nbr_idx] = local_bias
    bias_featT = np.ascontiguousarray(
        np.concatenate([rbf_ic, seqsep_ic, full_bias], axis=1).T)  # (128, N)

    # augmented, r-premultiplied h for the G matmul: [h | 32m | sd | pad]
    haug = np.zeros((N, AUG), f)
    haug[:, :D] = h * r64[:, None].astype(f)
    haug[:, D] = (32.0 * m64 * r64).astype(f)
    haug[:, D + 1] = 1.0                                # r*sd
    haug8 = haug.astype(f8)

    hT_full = np.ascontiguousarray(h.T)

    W1b = W1[D:]
    Wgb = Wg[D:]
    WA = (Wo.astype(np.float64) @ W1b.astype(np.float64)).astype(f)
    WG = (Wo.astype(np.float64) @ Wgb.astype(np.float64)).astype(f)
    a0c = (h[c].astype(np.float64) @ W1b.astype(np.float64) + b1).astype(f)
    g0c = (h[c].astype(np.float64) @ Wgb.astype(np.float64) + bg).astype(f)

    def wtile(x, dt):
        return np.ascontiguousarray(
            np.asarray(x, f).reshape(KT, 128, D).transpose(1, 0, 2)
            .reshape(128, KT * D)).astype(dt)

    gamma_a32 = gamma_a.astype(f)
    beta_a32 = beta_a.astype(f)
    Wb_pad = Wb
    shared = {
        "Wkp8t": np.ascontiguousarray(
            Wkp8.reshape(KT, 128, H).transpose(1, 0, 2).reshape(128, KT * H)),
        "Wb": Wb.astype(f8),
        "W1t8t": wtile(16.0 * W1[:D], f8),
        "Wgt8t": wtile(16.0 * Wg[:D], f8),
        "W2h8t": wtile(16.0 * RES * W2, f8),
        "Wvt": wtile(Wv, bf), "Wot": wtile(Wo, bf),
        "WAt": wtile(WA, bf), "WGt": wtile(WG, bf),
        "idn": np.eye(128, dtype=f),
        "idnb": np.eye(16, dtype=bf),
        "cbv": cbv,
        "gb16": np.ascontiguousarray(np.broadcast_to(gamma_a32, (H, D))),
        "bb16": np.ascontiguousarray(np.broadcast_to(beta_a32, (H, D))),
        "hcv": np.ascontiguousarray(h[c].reshape(KT, 128).T),
        "a0c": np.ascontiguousarray(a0c.reshape(KT, 128).T),
        "g0c": np.ascontiguousarray(g0c.reshape(KT, 128).T),
        "b2v": np.ascontiguousarray((RES * b2).reshape(KT, 128).T),
    }
    in_maps = []
    for i in range(NCORES):
        r0 = i * NS
        mm = dict(shared)
        hTs = hT_full[:, r0:r0 + NS]
        mm["hT8t"] = _tile_rows(hTs, NCH, CH).astype(f8)
        mm["hTbt"] = _tile_rows(hTs, NCH, CH).astype(bf)
        ha = haug8[r0:r0 + NS]            # (NS, AUG) fp8
        mm["hN8t"] = np.ascontiguousarray(
            ha.reshape(NCH, 4, 128, AUG).transpose(0, 2, 1, 3)
            .reshape(NCH * 128, 4 * AUG))
        mm["bT"] = np.ascontiguousarray(bias_featT[:, r0:r0 + NS]).astype(f8)
        rr = r64[r0:r0 + NS].astype(bf)
        mm["rb"] = np.ascontiguousarray(np.broadcast_to(rr, (H, NS)))
        mm["mncgb"] = (ncg256[:, None] * m64[None, r0:r0 + NS]).astype(bf)
        in_maps.append(mm)

    nc = _get_nc()
    trace = bool(int(os.environ.get("KERNEL_TRACE", "0")))
    res = run_bass_kernel_spmd(nc, in_maps, core_ids=list(range(NCORES)),
                               trace=trace)
    LAST_RESULTS = res

    out = np.empty((N, D), f)
    for i in range(NCORES):
        out[i * NS:(i + 1) * NS] = res.results[i]["outT"].T
    hcn = res.results[0]["outC"].T.reshape(D)           # [m,p] -> flat
    out[c] = hcn
    return out
